# revision 29
# baseline (speedup 1.0000x reference)
"""GCContext (global-context pooling) Trainium2 Bass kernel.

Problem (per sample): x [C=1024, HW=4096] fp32
  logits = (w @ x + b) / sqrt(C)        # [HW]
  attn   = softmax(logits)              # [HW]
  focus  = x @ attn                     # [C]
Output: [B, C, 1, 1].

Design (B=16 data-parallel over 8 cores, 2 samples/core, fp16 data path):
  - x and w are cast to fp16 on the host (halves HBM traffic; measured
    output error ~2e-4 relative). attn, Z, and all accumulations stay fp32.
  - x is streamed in 2MB pieces of 1024 spatial positions, host-permuted
    to [b, piece, partition, half, chunk, s] so each 1MB half-piece is one
    contiguous DMA with 8KB/partition runs; halves alternate between the
    two HWDGE rings (sync/scalar) so pieces complete in order.
  - PE computes logits with a replicated-w stationary [128,128], so each
    PSUM bank holds 512 logits broadcast across all 128 partitions - the
    partition-broadcast the vector engine needs for the focus multiply.
  - ACT does exp((logits+b)/32) from PSUM with fused per-partition
    Z-accumulation (accum_out). No max-subtraction: logits are
    ~N(0, 0.02^2) by construction, exp is safe; softmax is
    shift-invariant so results match the reference.
  - DVE does the focus contraction with fused scalar_tensor_tensor
    (x*attn, sum along free dim -> accum_out), one op per channel chunk
    per piece; first/last pieces use half-piece granularity to shorten
    pipeline fill and drain. DVE is the bottleneck engine (~80us busy,
    ~97% occupancy); DMA is ~60us, PE/ACT hide underneath.
  - Final normalize by 1/Z and store [128, 8] per sample; host reassembles.
"""

import sys


for _p in ("/opt/trn_rl_repo",):
    if _p not in sys.path:
        sys.path.insert(0, _p)

import numpy as np

import concourse.bacc as bacc
import concourse.tile as tile
from concourse import mybir
from concourse.bass_utils import run_bass_kernel_spmd

N_CORES = 8
B = 16
C = 1024
H = 64
W = 64
HW = H * W
B_LOC = B // N_CORES          # samples per core
R = C // 128                  # channel chunks (partition groups)
NS = 8                        # spatial slices per sample
S = HW // NS                  # spatial positions per slice (512)
SCALE = 1.0 / 32.0            # 1/sqrt(C)

_CACHE = {}


def _build_nc():
    nc = bacc.Bacc("TRN2", target_bir_lowering=False, debug=False,
                   num_devices=N_CORES)
    fp32 = mybir.dt.float32

    fp16 = mybir.dt.float16
    xs = nc.dram_tensor("xs", [B_LOC, NS // 2, 128, 2, R, S], fp16,
                        kind="ExternalInput")
    wrep = nc.dram_tensor("wrep", [128, R, 128], fp16, kind="ExternalInput")
    bias = nc.dram_tensor("bias", [128, 1], fp32, kind="ExternalInput")
    out = nc.dram_tensor("focus_out", [B_LOC, 128, R], fp32, kind="ExternalOutput")

    with tile.TileContext(nc) as tc:
        with (
            tc.tile_pool(name="singles", bufs=1) as singles,
            tc.tile_pool(name="xp", bufs=3) as xp,
            tc.tile_pool(name="attnp", bufs=2) as attnp,
            tc.tile_pool(name="accp", bufs=4) as accp,
            tc.tile_pool(name="scrp", bufs=2) as scrp,
            tc.tile_pool(name="smallp", bufs=4) as smallp,
            tc.tile_pool(name="psum", bufs=4, space="PSUM") as psump,
        ):
            w_sb = singles.tile([128, R, 128], fp16)
            nc.scalar.dma_start(out=w_sb[:], in_=wrep[:])
            bias_sb = singles.tile([128, 1], fp32)
            nc.scalar.dma_start(out=bias_sb[:], in_=bias[:])

            NJ = NS // 2
            SQ = S // 2               # quarter-piece width (256 positions)

            def do_unit(x_sub, attn_sub, z_sub, fp_sub, width, uid):
                """One logits->exp->focus unit over `width` positions.

                x_sub(r): [128, width] slice of x for channel chunk r
                attn_sub: [128, width] destination/operand slice of attn
                z_sub / fp_sub(r): [128, 1] accum slots
                """
                ps = psump.tile([128, width], fp32, name=f"ps{uid % 2}",
                                tag=f"ps{uid % 2}")
                for r in range(R):
                    nc.tensor.matmul(ps[:], lhsT=w_sb[:, r, :], rhs=x_sub(r),
                                     start=(r == 0), stop=(r == R - 1))
                nc.scalar.activation(attn_sub, ps[:],
                                     mybir.ActivationFunctionType.Exp,
                                     bias=bias_sb[:, 0:1], scale=SCALE,
                                     accum_out=z_sub)
                for r in range(R):
                    scr = scrp.tile([128, 2, S], fp32,
                                    name=f"scr{r % 2}", tag=f"scr{r % 2}")
                    nc.vector.scalar_tensor_tensor(
                        out=scr[:, 0, :width],
                        in0=x_sub(r),
                        scalar=1.0,
                        in1=attn_sub,
                        op0=mybir.AluOpType.mult,
                        op1=mybir.AluOpType.mult,
                        accum_out=fp_sub(r))

            for b in range(B_LOC):
                attn_t = attnp.tile([128, NS, S], fp32)
                fparts = accp.tile([128, R, NJ], fp32)
                # NS half-piece slots + 4 quarter slots (head/tail pieces)
                zpart = accp.tile([128, NS + 4], fp32)
                nc.vector.memset(zpart[:], 0.0)
                for j in range(NJ):
                    x_t = xp.tile([128, 2, R, S], fp16)
                    head = (b == 0 and j == 0)
                    tail = (b == B_LOC - 1 and j == NJ - 1)
                    if head:
                        # quarter DMAs on both rings: first unit is runnable
                        # after 0.5MB, shortening pipeline fill
                        nc.sync.dma_start(out=x_t[:, 0, :, 0:SQ],
                                          in_=xs[b, j, :, 0, :, 0:SQ])
                        nc.scalar.dma_start(out=x_t[:, 0, :, SQ:S],
                                            in_=xs[b, j, :, 0, :, SQ:S])
                        nc.sync.dma_start(out=x_t[:, 1], in_=xs[b, j, :, 1])
                    elif tail:
                        # last 1MB arrives as two quarters so the post-DMA
                        # chain is one quarter of MMs + exp + FD=256 reduces
                        nc.scalar.dma_start(out=x_t[:, 0], in_=xs[b, j, :, 0])
                        nc.sync.dma_start(out=x_t[:, 1, :, 0:SQ],
                                          in_=xs[b, j, :, 1, :, 0:SQ])
                        nc.sync.dma_start(out=x_t[:, 1, :, SQ:S],
                                          in_=xs[b, j, :, 1, :, SQ:S])
                    else:
                        # halves alternate HWDGE rings: pieces complete in
                        # order, matmuls start after the first 1MB
                        nc.sync.dma_start(out=x_t[:, 0], in_=xs[b, j, :, 0])
                        nc.scalar.dma_start(out=x_t[:, 1], in_=xs[b, j, :, 1])

                    if head or tail:
                        fpq = accp.tile([128, R, 3], fp32,
                                        name="fpq", tag="fpq")
                        if head:
                            units = [(0, 0, SQ, NS), (0, SQ, S, NS + 1),
                                     (1, 0, S, 2 * j + 1)]
                        else:
                            units = [(0, 0, S, 2 * j), (1, 0, SQ, NS + 2),
                                     (1, SQ, S, NS + 3)]
                        for u, (k, s0, s1, zc) in enumerate(units):
                            h = 2 * j + k
                            do_unit(
                                lambda r, k=k, s0=s0, s1=s1:
                                    x_t[:, k, r, s0:s1],
                                attn_t[:, h, s0:s1],
                                zpart[:, zc:zc + 1],
                                lambda r, u=u: fpq[:, r, u:u + 1],
                                s1 - s0, u)
                        for r in range(R):
                            nc.vector.tensor_reduce(
                                fparts[:, r, j:j + 1], fpq[:, r, :],
                                axis=mybir.AxisListType.X,
                                op=mybir.AluOpType.add)
                    else:
                        ps = [psump.tile([128, S], fp32, name=f"ps{k}",
                                         tag=f"ps{k}")
                              for k in range(2)]
                        # r-outer: one stationary load per channel chunk
                        for r in range(R):
                            for k in range(2):
                                nc.tensor.matmul(
                                    ps[k][:],
                                    lhsT=w_sb[:, r, :],
                                    rhs=x_t[:, k, r, :],
                                    start=(r == 0), stop=(r == R - 1))
                        for k in range(2):
                            h = 2 * j + k
                            nc.scalar.activation(
                                attn_t[:, h, :], ps[k][:],
                                mybir.ActivationFunctionType.Exp,
                                bias=bias_sb[:, 0:1], scale=SCALE,
                                accum_out=zpart[:, h:h + 1])
                        for r in range(R):
                            scr = scrp.tile([128, 2, S], fp32,
                                            name=f"scr{r % 2}",
                                            tag=f"scr{r % 2}")
                            nc.vector.scalar_tensor_tensor(
                                out=scr[:],
                                in0=x_t[:, :, r, :],
                                scalar=1.0,
                                in1=attn_t[:, 2 * j:2 * j + 2, :],
                                op0=mybir.AluOpType.mult,
                                op1=mybir.AluOpType.mult,
                                accum_out=fparts[:, r, j:j + 1])
                ztot = smallp.tile([128, 1], fp32)
                nc.vector.tensor_reduce(ztot[:], zpart[:],
                                        axis=mybir.AxisListType.X,
                                        op=mybir.AluOpType.add)
                rz = smallp.tile([128, 1], fp32)
                nc.vector.reciprocal(rz[:], ztot[:])
                facc = smallp.tile([128, R], fp32)
                nc.vector.tensor_reduce(facc[:], fparts[:],
                                        axis=mybir.AxisListType.X,
                                        op=mybir.AluOpType.add)
                fout = smallp.tile([128, R], fp32)
                nc.vector.tensor_scalar_mul(fout[:], facc[:], rz[:, 0:1])
                nc.sync.dma_start(out=out[b], in_=fout[:])

    nc.compile()
    return nc


def _get_nc():
    if "nc" not in _CACHE:
        _CACHE["nc"] = _build_nc()
    return _CACHE["nc"]


def _prep_core_inputs(x, key_w, key_b):
    """Build the per-core input maps (host-side shard + layout permute)."""
    # [B, C, H, W] -> [B, R, 128, NS/2, 2, S] -> [B, NS/2, 128, 2, R, S]
    xv = np.ascontiguousarray(
        x.reshape(B, R, 128, NS // 2, 2, S).transpose(0, 3, 2, 4, 1, 5)
    ).astype(np.float16)
    wrep = np.ascontiguousarray(
        np.broadcast_to(key_w.reshape(R, 128).T[:, :, None], (128, R, 128))
    ).astype(np.float16)
    bias = np.full((128, 1), key_b[0] * SCALE, dtype=np.float32)
    in_maps = []
    for c in range(N_CORES):
        in_maps.append({
            "xs": xv[c * B_LOC:(c + 1) * B_LOC],
            "wrep": wrep,
            "bias": bias,
        })
    return in_maps


def kernel(x, key_w, key_b):
    x = np.asarray(x, dtype=np.float32)
    key_w = np.asarray(key_w, dtype=np.float32)
    key_b = np.asarray(key_b, dtype=np.float32)
    assert x.shape == (B, C, H, W), x.shape

    nc = _get_nc()
    in_maps = _prep_core_inputs(x, key_w, key_b)
    res = run_bass_kernel_spmd(nc, in_maps, list(range(N_CORES)))

    out = np.empty((B, C), dtype=np.float32)
    for c in range(N_CORES):
        f = res.results[c]["focus_out"]          # [B_LOC, 128, R]
        out[c * B_LOC:(c + 1) * B_LOC] = (
            f.transpose(0, 2, 1).reshape(B_LOC, C))
    return out.reshape(B, C, 1, 1)


# revision 30
# speedup vs baseline: 1.0080x; 1.0080x over previous
"""GCContext (global-context pooling) Trainium2 Bass kernel.

Problem (per sample): x [C=1024, HW=4096] fp32
  logits = (w @ x + b) / sqrt(C)        # [HW]
  attn   = softmax(logits)              # [HW]
  focus  = x @ attn                     # [C]
Output: [B, C, 1, 1].

Design (B=16 data-parallel over 8 cores, 2 samples/core, fp16 data path):
  - x and w are cast to fp16 on the host (halves HBM traffic; measured
    output error ~2e-4 relative). attn, Z, and all accumulations stay fp32.
  - x is streamed in 2MB pieces of 1024 spatial positions, host-permuted
    to [b, piece, partition, half, chunk, s] so each 1MB half-piece is one
    contiguous DMA with 8KB/partition runs; halves alternate between the
    two HWDGE rings (sync/scalar) so pieces complete in order.
  - PE computes logits with a replicated-w stationary [128,128], so each
    PSUM bank holds 512 logits broadcast across all 128 partitions - the
    partition-broadcast the vector engine needs for the focus multiply.
  - ACT does exp((logits+b)/32) from PSUM with fused per-partition
    Z-accumulation (accum_out). No max-subtraction: logits are
    ~N(0, 0.02^2) by construction, exp is safe; softmax is
    shift-invariant so results match the reference.
  - DVE does the focus contraction with fused scalar_tensor_tensor
    (x*attn, sum along free dim -> accum_out), one op per channel chunk
    per piece; first/last pieces use half-piece granularity to shorten
    pipeline fill and drain. DVE is the bottleneck engine (~80us busy,
    ~97% occupancy); DMA is ~60us, PE/ACT hide underneath.
  - Final normalize by 1/Z and store [128, 8] per sample; host reassembles.
"""

import sys


for _p in ("/opt/trn_rl_repo",):
    if _p not in sys.path:
        sys.path.insert(0, _p)

import numpy as np

import concourse.bacc as bacc
import concourse.tile as tile
from concourse import mybir
from concourse.bass_utils import run_bass_kernel_spmd

N_CORES = 8
B = 16
C = 1024
H = 64
W = 64
HW = H * W
B_LOC = B // N_CORES          # samples per core
R = C // 128                  # channel chunks (partition groups)
NS = 8                        # spatial slices per sample
S = HW // NS                  # spatial positions per slice (512)
SCALE = 1.0 / 32.0            # 1/sqrt(C)

_CACHE = {}


def _build_nc():
    nc = bacc.Bacc("TRN2", target_bir_lowering=False, debug=False,
                   num_devices=N_CORES)
    fp32 = mybir.dt.float32

    fp16 = mybir.dt.float16
    xs = nc.dram_tensor("xs", [B_LOC, NS // 2, 128, 2, R, S], fp16,
                        kind="ExternalInput")
    wrep = nc.dram_tensor("wrep", [128, R, 128], fp16, kind="ExternalInput")
    bias = nc.dram_tensor("bias", [128, 1], fp32, kind="ExternalInput")
    out = nc.dram_tensor("focus_out", [B_LOC, 128, R], fp32, kind="ExternalOutput")

    with tile.TileContext(nc) as tc:
        with (
            tc.tile_pool(name="singles", bufs=1) as singles,
            tc.tile_pool(name="xp", bufs=3) as xp,
            tc.tile_pool(name="attnp", bufs=2) as attnp,
            tc.tile_pool(name="accp", bufs=4) as accp,
            tc.tile_pool(name="scrp", bufs=2) as scrp,
            tc.tile_pool(name="smallp", bufs=4) as smallp,
            tc.tile_pool(name="psum", bufs=4, space="PSUM") as psump,
        ):
            w_sb = singles.tile([128, R, 128], fp16)
            nc.scalar.dma_start(out=w_sb[:], in_=wrep[:])
            bias_sb = singles.tile([128, 1], fp32)
            nc.scalar.dma_start(out=bias_sb[:], in_=bias[:])

            NJ = NS // 2
            SQ = S // 2               # quarter-piece width (256 positions)

            def do_unit(x_sub, attn_sub, z_sub, fp_sub, width, uid):
                """One logits->exp->focus unit over `width` positions.

                x_sub(r): [128, width] slice of x for channel chunk r
                attn_sub: [128, width] destination/operand slice of attn
                z_sub / fp_sub(r): [128, 1] accum slots
                """
                ps = psump.tile([128, width], fp32, name=f"ps{uid % 2}",
                                tag=f"ps{uid % 2}")
                for r in range(R):
                    nc.tensor.matmul(ps[:], lhsT=w_sb[:, r, :], rhs=x_sub(r),
                                     start=(r == 0), stop=(r == R - 1))
                nc.scalar.activation(attn_sub, ps[:],
                                     mybir.ActivationFunctionType.Exp,
                                     bias=bias_sb[:, 0:1], scale=SCALE,
                                     accum_out=z_sub)
                for r in range(R):
                    scr = scrp.tile([128, 2, S], fp32,
                                    name=f"scr{r % 2}", tag=f"scr{r % 2}")
                    nc.vector.scalar_tensor_tensor(
                        out=scr[:, 0, :width],
                        in0=x_sub(r),
                        scalar=1.0,
                        in1=attn_sub,
                        op0=mybir.AluOpType.mult,
                        op1=mybir.AluOpType.mult,
                        accum_out=fp_sub(r))

            for b in range(B_LOC):
                attn_t = attnp.tile([128, NS, S], fp32)
                fparts = accp.tile([128, R, NJ], fp32)
                # NS half-piece slots + 3 sub-piece slots (head piece)
                zpart = accp.tile([128, NS + 3], fp32)
                nc.vector.memset(zpart[:], 0.0)
                for j in range(NJ):
                    x_t = xp.tile([128, 2, R, S], fp16)
                    head = (b == 0 and j == 0)
                    if head:
                        # eighth/quarter DMAs across both rings: the first
                        # unit is runnable after 0.25MB, shortening the fill
                        # until the (saturated) vector engine starts
                        SE = SQ // 2
                        nc.sync.dma_start(out=x_t[:, 0, :, 0:SE],
                                          in_=xs[b, j, :, 0, :, 0:SE])
                        nc.scalar.dma_start(out=x_t[:, 0, :, SE:SQ],
                                            in_=xs[b, j, :, 0, :, SE:SQ])
                        nc.scalar.dma_start(out=x_t[:, 0, :, SQ:S],
                                            in_=xs[b, j, :, 0, :, SQ:S])
                        nc.sync.dma_start(out=x_t[:, 1], in_=xs[b, j, :, 1])
                    else:
                        # halves alternate HWDGE rings: pieces complete in
                        # order, matmuls start after the first 1MB
                        nc.sync.dma_start(out=x_t[:, 0], in_=xs[b, j, :, 0])
                        nc.scalar.dma_start(out=x_t[:, 1], in_=xs[b, j, :, 1])

                    if head:
                        SE = SQ // 2
                        fpq = accp.tile([128, R, 4], fp32,
                                        name="fpq", tag="fpq")
                        units = [(0, 0, SE, NS), (0, SE, SQ, NS + 1),
                                 (0, SQ, S, NS + 2), (1, 0, S, 2 * j + 1)]
                        for u, (k, s0, s1, zc) in enumerate(units):
                            h = 2 * j + k
                            do_unit(
                                lambda r, k=k, s0=s0, s1=s1:
                                    x_t[:, k, r, s0:s1],
                                attn_t[:, h, s0:s1],
                                zpart[:, zc:zc + 1],
                                lambda r, u=u: fpq[:, r, u:u + 1],
                                s1 - s0, u)
                        for r in range(R):
                            nc.vector.tensor_reduce(
                                fparts[:, r, j:j + 1], fpq[:, r, :],
                                axis=mybir.AxisListType.X,
                                op=mybir.AluOpType.add)
                    else:
                        ps = [psump.tile([128, S], fp32, name=f"ps{k}",
                                         tag=f"ps{k}")
                              for k in range(2)]
                        # r-outer: one stationary load per channel chunk
                        for r in range(R):
                            for k in range(2):
                                nc.tensor.matmul(
                                    ps[k][:],
                                    lhsT=w_sb[:, r, :],
                                    rhs=x_t[:, k, r, :],
                                    start=(r == 0), stop=(r == R - 1))
                        for k in range(2):
                            h = 2 * j + k
                            nc.scalar.activation(
                                attn_t[:, h, :], ps[k][:],
                                mybir.ActivationFunctionType.Exp,
                                bias=bias_sb[:, 0:1], scale=SCALE,
                                accum_out=zpart[:, h:h + 1])
                        for r in range(R):
                            scr = scrp.tile([128, 2, S], fp32,
                                            name=f"scr{r % 2}",
                                            tag=f"scr{r % 2}")
                            nc.vector.scalar_tensor_tensor(
                                out=scr[:],
                                in0=x_t[:, :, r, :],
                                scalar=1.0,
                                in1=attn_t[:, 2 * j:2 * j + 2, :],
                                op0=mybir.AluOpType.mult,
                                op1=mybir.AluOpType.mult,
                                accum_out=fparts[:, r, j:j + 1])
                ztot = smallp.tile([128, 1], fp32)
                nc.vector.tensor_reduce(ztot[:], zpart[:],
                                        axis=mybir.AxisListType.X,
                                        op=mybir.AluOpType.add)
                rz = smallp.tile([128, 1], fp32)
                nc.vector.reciprocal(rz[:], ztot[:])
                facc = smallp.tile([128, R], fp32)
                nc.vector.tensor_reduce(facc[:], fparts[:],
                                        axis=mybir.AxisListType.X,
                                        op=mybir.AluOpType.add)
                fout = smallp.tile([128, R], fp32)
                nc.vector.tensor_scalar_mul(fout[:], facc[:], rz[:, 0:1])
                nc.sync.dma_start(out=out[b], in_=fout[:])

    nc.compile()
    return nc


def _get_nc():
    if "nc" not in _CACHE:
        _CACHE["nc"] = _build_nc()
    return _CACHE["nc"]


def _prep_core_inputs(x, key_w, key_b):
    """Build the per-core input maps (host-side shard + layout permute)."""
    # [B, C, H, W] -> [B, R, 128, NS/2, 2, S] -> [B, NS/2, 128, 2, R, S]
    xv = np.ascontiguousarray(
        x.reshape(B, R, 128, NS // 2, 2, S).transpose(0, 3, 2, 4, 1, 5)
    ).astype(np.float16)
    wrep = np.ascontiguousarray(
        np.broadcast_to(key_w.reshape(R, 128).T[:, :, None], (128, R, 128))
    ).astype(np.float16)
    bias = np.full((128, 1), key_b[0] * SCALE, dtype=np.float32)
    in_maps = []
    for c in range(N_CORES):
        in_maps.append({
            "xs": xv[c * B_LOC:(c + 1) * B_LOC],
            "wrep": wrep,
            "bias": bias,
        })
    return in_maps


def kernel(x, key_w, key_b):
    x = np.asarray(x, dtype=np.float32)
    key_w = np.asarray(key_w, dtype=np.float32)
    key_b = np.asarray(key_b, dtype=np.float32)
    assert x.shape == (B, C, H, W), x.shape

    nc = _get_nc()
    in_maps = _prep_core_inputs(x, key_w, key_b)
    res = run_bass_kernel_spmd(nc, in_maps, list(range(N_CORES)))

    out = np.empty((B, C), dtype=np.float32)
    for c in range(N_CORES):
        f = res.results[c]["focus_out"]          # [B_LOC, 128, R]
        out[c * B_LOC:(c + 1) * B_LOC] = (
            f.transpose(0, 2, 1).reshape(B_LOC, C))
    return out.reshape(B, C, 1, 1)
